# revision 1
# baseline (speedup 1.0000x reference)
"""Trainium2 Bass kernel for AllegroScalarOutputHead (segment_reduce).

Strategy (8 NeuronCores, SPMD, no collectives):
  - Graphs 4k..4k+3 -> core k (batch is sorted, so each core owns a contiguous
    node range [n0, n1)). Edges go to the core that owns their TARGET node.
  - Features are shipped transposed (feature-major) so the MLP matmuls need no
    on-device transpose; fp32 data is fed to the PE as float32r (1 cyc/row).
  - Per-edge pair coefficient c_e = pair_scales[zs*101+zt] * atom_scales[zt]
    is realized on device via three indirect-DMA gathers (zs, zt from the
    replicated atomic-number table; c from a device-built fused G table).
  - Per-graph reduction: cumulative boundary one-hots (is_lt vs the 4 graph
    boundaries) contracted against per-edge/per-node energies on the PE into
    a single PSUM accumulator [4,1] per core; host un-diffs and concatenates.
"""

import numpy as np

NCORES = 8
N_NODES = 50000
NUM_GRAPHS = 32
NZ = 101            # atomic number table entries (0..100)
D_EDGE = 128
D_NODE = 256
SUPER = 512         # edge mlp supertile (free dim)
EDGE_BLOCK = 4096   # edges per DMA/compute block
NODE_PAIR = 256     # node mlp processes 256 nodes (2 tiles) per matmul
GTAB = 10304        # padded fused-pair-table size (>= 101*101 + 101 + 1)
SENT_NODE = N_NODES  # sentinel node id (Z = 101 -> kills pad edges/nodes)

_CACHE = {}


def _build_edge(ET, NT, act="silu"):
    """Build the SPMD bass program for per-core shard sizes ET (edges, mult of
    EDGE_BLOCK) and NT (nodes, mult of NODE_PAIR). Returns compiled nc."""
    import concourse.bass as bass
    import concourse.tile as tile
    from concourse import bacc, mybir
    from concourse.bass import IndirectOffsetOnAxis
    from contextlib import ExitStack

    f32 = mybir.dt.float32
    f32r = mybir.dt.float32r
    bf16 = mybir.dt.bfloat16
    i32 = mybir.dt.int32
    AF = mybir.ActivationFunctionType
    OP = mybir.AluOpType
    AFUNC = AF.Silu if act == "silu" else AF.Sigmoid

    EC = ET // 128          # columns of per-edge scalars
    NTC = NT // 128         # columns of per-node scalars
    NBLK = ET // EDGE_BLOCK
    CPB = EDGE_BLOCK // 128  # pe/idx columns per edge block (32)
    n_y_mm = NTC + EC       # total Y-accumulation matmuls

    nc = bacc.Bacc("TRN2", debug=False, num_devices=NCORES)

    # ---------------- DRAM parameters (per-core shards / replicated) --------
    eT = nc.declare_dram_parameter("eT", [D_EDGE, ET], f32, isOutput=False)
    isw = nc.declare_dram_parameter("isw", [128, EC], i32, isOutput=False)
    itw = nc.declare_dram_parameter("itw", [128, EC], i32, isOutput=False)
    Zext = nc.declare_dram_parameter("Zext", [N_NODES + 1], i32, isOutput=False)
    ascale = nc.declare_dram_parameter("ascale", [NZ + 1], f32, isOutput=False)
    ashift = nc.declare_dram_parameter("ashift", [NZ + 1], f32, isOutput=False)
    pair = nc.declare_dram_parameter("pair", [NZ, NZ], f32, isOutput=False)
    iotaR_d = nc.declare_dram_parameter("iotaR", [128, NZ + 1], i32, isOutput=False)
    W1e_d = nc.declare_dram_parameter("W1e", [128, 128], f32, isOutput=False)
    b1e_d = nc.declare_dram_parameter("b1e", [128, 1], f32, isOutput=False)
    W2e_d = nc.declare_dram_parameter("W2e", [128, 1], f32, isOutput=False)
    b2_d = nc.declare_dram_parameter("b2", [128, 2], f32, isOutput=False)  # [b2e, b2n] replicated
    Brow_d = nc.declare_dram_parameter("Brow", [128, 4], i32, isOutput=False)   # global, replicated
    out_d = nc.declare_dram_parameter("out", [1, 4], f32, isOutput=True)

    Gdram = nc.dram_tensor("Gdram", [GTAB], f32)

    with tile.TileContext(nc) as tc, ExitStack() as ctx:
        const = ctx.enter_context(tc.tile_pool(name="const", bufs=1))
        edgep = ctx.enter_context(tc.tile_pool(name="edgep", bufs=3))
        hep = ctx.enter_context(tc.tile_pool(name="hep", bufs=3))
        nodep = ctx.enter_context(tc.tile_pool(name="nodep", bufs=2))
        smallp = ctx.enter_context(tc.tile_pool(name="smallp", bufs=2))
        ps_mm1 = ctx.enter_context(tc.tile_pool(name="ps_mm1", bufs=2, space="PSUM"))
        ps_pe = ctx.enter_context(tc.tile_pool(name="ps_pe", bufs=2, space="PSUM"))
        ps_acc = ctx.enter_context(tc.tile_pool(name="ps_acc", bufs=1, space="PSUM"))

        # ---------------- phase 0: constants -------------------------------
        W1e = const.tile([128, 128], f32)
        nc.sync.dma_start(W1e[:], W1e_d.ap())
        b1e = const.tile([128, 1], f32)
        nc.sync.dma_start(b1e[:], b1e_d.ap())
        W2e = const.tile([128, 1], f32)
        nc.sync.dma_start(W2e[:], W2e_d.ap())
        b2 = const.tile([128, 2], f32)
        nc.sync.dma_start(b2[:], b2_d.ap())
        Brow = const.tile([128, 4], i32)
        nc.sync.dma_start(Brow[:], Brow_d.ap())
        # fused pair table, stored transposed: G[b, a] = pair[a, b] * ascale[b]
        # ("pair" param is shipped transposed by the host). Indexed zt*101+zs.
        pair_s = const.tile([NZ, NZ], f32)
        nc.sync.dma_start(pair_s[:], pair.ap())
        asc_col = const.tile([NZ, 1], f32)
        nc.sync.dma_start(
            asc_col[:], ascale.ap()[0:NZ].rearrange("(a b) -> a b", b=1)
        )
        G_s = const.tile([NZ, NZ], f32)
        nc.vector.tensor_scalar(G_s[:], pair_s[:], asc_col[:], None, OP.mult)
        nc.sync.dma_start(
            Gdram.ap()[0:NZ * NZ].rearrange("(a b) -> a b", a=NZ), G_s[:]
        )
        zrow = const.tile([1, GTAB - NZ * NZ], f32)
        nc.vector.memset(zrow[:], 0.0)
        nc.sync.dma_start(
            Gdram.ap()[NZ * NZ:GTAB].rearrange("(a b) -> a b", a=1), zrow[:]
        )

        iotaR = const.tile([128, NZ + 1], i32)
        nc.sync.dma_start(iotaR[:], iotaR_d.ap())
        ones_col = const.tile([NZ, 1], f32)
        nc.vector.memset(ones_col[:], 1.0)

        # per-edge index arrays + z gathers (front-loaded, chunked in halves)
        isw_s = const.tile([128, EC], i32)
        nc.sync.dma_start(isw_s[:], isw.ap())
        itw_s = const.tile([128, EC], i32)
        nc.sync.dma_start(itw_s[:], itw.ap())
        zs_s = const.tile([128, EC], i32)
        zt_s = const.tile([128, EC], i32)
        Zext2 = Zext.ap().rearrange("(a b) -> a b", b=1)
        for j in range(EC):
            nc.gpsimd.indirect_dma_start(
                zs_s[:, j:j + 1], None, Zext2,
                IndirectOffsetOnAxis(ap=isw_s[:, j:j + 1], axis=0),
            )
            nc.gpsimd.indirect_dma_start(
                zt_s[:, j:j + 1], None, Zext2,
                IndirectOffsetOnAxis(ap=itw_s[:, j:j + 1], axis=0),
            )

        Ye_ps = ps_acc.tile([1, 4], f32, tag="ye")

        # ---------------- phase 2: edge MLP + pair gather + reduce ---------
        NZ1 = NZ + 1
        Kps = ps_acc.tile([NZ1, 4 * NZ1], f32, tag="K")
        k_i = 0
        for b in range(NBLK):
            bcols = slice(b * CPB, (b + 1) * CPB)

            xe = edgep.tile([128, EDGE_BLOCK], f32, tag="xe")
            nc.sync.dma_start(xe[:], eT.ap()[:, b * EDGE_BLOCK:(b + 1) * EDGE_BLOCK])
            pe_ps = ps_pe.tile([128, CPB], f32, tag="pe")
            for s in range(EDGE_BLOCK // SUPER):
                ps = ps_mm1.tile([128, SUPER], f32, tag="mm1")
                nc.tensor.matmul(
                    ps[:], W1e[:], xe[:, s * SUPER:(s + 1) * SUPER],
                    start=True, stop=True,
                )
                he = hep.tile([128, SUPER], f32, tag="he_edge")
                nc.scalar.activation(he[:], ps[:], AFUNC, bias=b1e[:])
                for t in range(SUPER // 128):
                    col = s * (SUPER // 128) + t
                    nc.tensor.matmul(
                        pe_ps[:, col:col + 1],
                        he[:, t * 128:(t + 1) * 128], W2e[:],
                        start=True, stop=True,
                    )

            # w' = pe + b2e
            wp = smallp.tile([128, CPB], f32, tag="w")
            nc.vector.tensor_scalar(wp[:], pe_ps[:], b2[:, 0:1], None, OP.add)
            CUM4 = smallp.tile([128, CPB, 4], f32, tag="ecum")
            nc.vector.tensor_tensor(
                CUM4[:],
                itw_s[:, bcols].unsqueeze(2).broadcast_to([128, CPB, 4]),
                Brow[:].unsqueeze(1).broadcast_to([128, CPB, 4]),
                OP.is_lt,
            )
            SB = 8
            for sb in range(CPB // SB):
                scols = slice(b * CPB + sb * SB, b * CPB + (sb + 1) * SB)
                lcols = slice(sb * SB, (sb + 1) * SB)
                TOH = smallp.tile([128, SB, NZ1], f32, tag="toh")
                nc.vector.tensor_tensor(
                    TOH[:],
                    zt_s[:, scols].unsqueeze(2).broadcast_to([128, SB, NZ1]),
                    iotaR[:, 0:NZ1].unsqueeze(1).broadcast_to([128, SB, NZ1]),
                    OP.is_equal,
                )
                TOHW = smallp.tile([128, SB, NZ1], f32, tag="tohw")
                nc.vector.tensor_tensor(
                    TOHW[:], TOH[:],
                    wp[:, lcols].unsqueeze(2).broadcast_to([128, SB, NZ1]),
                    OP.mult,
                )
                USOH = smallp.tile([128, SB, NZ1], f32, tag="usoh")
                nc.vector.tensor_tensor(
                    USOH[:],
                    zs_s[:, scols].unsqueeze(2).broadcast_to([128, SB, NZ1]),
                    iotaR[:, 0:NZ1].unsqueeze(1).broadcast_to([128, SB, NZ1]),
                    OP.is_equal,
                )
                USOHG = smallp.tile([128, SB, 4, NZ1], f32, tag="usohg")
                nc.vector.tensor_tensor(
                    USOHG[:],
                    USOH[:].unsqueeze(2).broadcast_to([128, SB, 4, NZ1]),
                    CUM4[:, lcols, :].unsqueeze(3).broadcast_to([128, SB, 4, NZ1]),
                    OP.mult,
                )
                for j in range(SB):
                    nc.tensor.matmul(
                        Kps[:], TOHW[:, j, :],
                        USOHG[:, j, :, :].rearrange("p a b -> p (a b)"),
                        start=(k_i == 0), stop=(k_i == EC - 1),
                    )
                    k_i += 1

        assert k_i == EC
        # Y'_g = sum_{b,a} G[b, a] * K[b, g*NZ1 + a]   (a,b in [0,101))
        for g in range(4):
            GK = smallp.tile([NZ, NZ], f32, tag="gk")
            nc.vector.tensor_tensor(
                GK[:], G_s[:], Kps[0:NZ, g * NZ1:g * NZ1 + NZ], OP.mult,
            )
            GKc = smallp.tile([NZ, 1], f32, tag="gkc")
            nc.vector.tensor_reduce(GKc[:], GK[:], mybir.AxisListType.X, OP.add)
            nc.tensor.matmul(
                Ye_ps[:, g:g + 1], GKc[:], ones_col[:],
                start=True, stop=True,
            )
        ysb = const.tile([1, 4], f32)
        nc.vector.tensor_copy(ysb[:], Ye_ps[:])
        nc.sync.dma_start(out_d.ap(), ysb[:])

    nc.compile()
    return nc




def _build_node(NT, act="silu"):
    """Standalone node-side program (isolated from the edge gather storm)."""
    import concourse.tile as tile
    from concourse import bacc, mybir
    from contextlib import ExitStack

    f32 = mybir.dt.float32
    bf16 = mybir.dt.bfloat16
    i32 = mybir.dt.int32
    AF = mybir.ActivationFunctionType
    OP = mybir.AluOpType
    AFUNC = AF.Silu if act == "silu" else AF.Sigmoid
    NTC = NT // 128

    nc = bacc.Bacc("TRN2", debug=False, num_devices=NCORES)
    nTa = nc.declare_dram_parameter("nTa", [128, NT], f32, isOutput=False)
    nTb = nc.declare_dram_parameter("nTb", [128, NT], f32, isOutput=False)
    Znd = nc.declare_dram_parameter("Zn", [128, NTC], i32, isOutput=False)
    ascR_d = nc.declare_dram_parameter("ascR", [128, NZ + 1], f32, isOutput=False)
    ashR_d = nc.declare_dram_parameter("ashR", [128, NZ + 1], f32, isOutput=False)
    iotaR_d = nc.declare_dram_parameter("iotaR", [128, NZ + 1], i32, isOutput=False)
    W1n_d = nc.declare_dram_parameter("W1n", [256, 256], f32, isOutput=False)
    b1n_d = nc.declare_dram_parameter("b1n", [128, 2], f32, isOutput=False)
    W2n_d = nc.declare_dram_parameter("W2n", [128, 2], f32, isOutput=False)
    b2_d = nc.declare_dram_parameter("b2", [128, 2], f32, isOutput=False)
    idn_d = nc.declare_dram_parameter("idn", [128, NTC], i32, isOutput=False)
    BrowL_d = nc.declare_dram_parameter("BrowL", [128, 4], i32, isOutput=False)
    out_d = nc.declare_dram_parameter("out", [4, 1], f32, isOutput=True)

    with tile.TileContext(nc) as tc, ExitStack() as ctx:
        const = ctx.enter_context(tc.tile_pool(name="const", bufs=1))
        nodep = ctx.enter_context(tc.tile_pool(name="nodep", bufs=2))
        ps_node = ctx.enter_context(tc.tile_pool(name="ps_node", bufs=2, space="PSUM"))
        ps_pa = ctx.enter_context(tc.tile_pool(name="ps_pa", bufs=2, space="PSUM"))
        ps_acc = ctx.enter_context(tc.tile_pool(name="ps_acc", bufs=1, space="PSUM"))

        W1n = []
        for kb in range(2):
            for db in range(2):
                t = const.tile([128, 128], f32, tag=f"w1n{kb}{db}")
                nc.sync.dma_start(
                    t[:], W1n_d.ap()[kb * 128:(kb + 1) * 128, db * 128:(db + 1) * 128]
                )
                W1n.append(t)
        b1n = const.tile([128, 2], f32)
        nc.sync.dma_start(b1n[:], b1n_d.ap())
        W2n = const.tile([128, 2], f32)
        nc.sync.dma_start(W2n[:], W2n_d.ap())
        b2 = const.tile([128, 2], f32)
        nc.sync.dma_start(b2[:], b2_d.ap())
        BrowL = const.tile([128, 4], i32)
        nc.sync.dma_start(BrowL[:], BrowL_d.ap())
        idblk = const.tile([128, NTC], i32)
        nc.sync.dma_start(idblk[:], idn_d.ap())
        Zn_s = const.tile([128, NTC], i32)
        nc.sync.dma_start(Zn_s[:], Znd.ap())
        ascR = const.tile([128, NZ + 1], f32)
        nc.sync.dma_start(ascR[:], ascR_d.ap())
        ashR = const.tile([128, NZ + 1], f32)
        nc.sync.dma_start(ashR[:], ashR_d.ap())
        iotaR = const.tile([128, NZ + 1], i32)
        nc.sync.dma_start(iotaR[:], iotaR_d.ap())

        NOH = const.tile([128, NTC, NZ + 1], bf16)
        nc.vector.tensor_tensor(
            NOH[:],
            Zn_s[:].unsqueeze(2).broadcast_to([128, NTC, NZ + 1]),
            iotaR[:].unsqueeze(1).broadcast_to([128, NTC, NZ + 1]),
            OP.is_equal,
        )
        sNp = const.tile([128, NTC, NZ + 1], f32)
        nc.vector.tensor_tensor(
            sNp[:], NOH[:],
            ascR[:].unsqueeze(1).broadcast_to([128, NTC, NZ + 1]), OP.mult,
        )
        sN = const.tile([128, NTC], f32)
        nc.vector.tensor_reduce(
            sN[:].unsqueeze(2), sNp[:], mybir.AxisListType.X, OP.add,
        )
        hNp = const.tile([128, NTC, NZ + 1], f32)
        nc.vector.tensor_tensor(
            hNp[:], NOH[:],
            ashR[:].unsqueeze(1).broadcast_to([128, NTC, NZ + 1]), OP.mult,
        )
        hN = const.tile([128, NTC], f32)
        nc.vector.tensor_reduce(
            hN[:].unsqueeze(2), hNp[:], mybir.AxisListType.X, OP.add,
        )

        nTa_s = const.tile([128, NT], f32)
        nc.sync.dma_start(nTa_s[:], nTa.ap())
        nTb_s = const.tile([128, NT], f32)
        nc.sync.dma_start(nTb_s[:], nTb.ap())

        pa_sb = const.tile([128, NTC], f32)
        for jp in range(NT // NODE_PAIR):
            cols = slice(jp * NODE_PAIR, (jp + 1) * NODE_PAIR)
            hes = []
            for db in range(2):
                ps = ps_node.tile([128, NODE_PAIR], f32, tag="ps_node")
                nc.tensor.matmul(
                    ps[:], W1n[0 * 2 + db][:], nTa_s[:, cols],
                    start=True, stop=False,
                )
                nc.tensor.matmul(
                    ps[:], W1n[1 * 2 + db][:], nTb_s[:, cols],
                    start=False, stop=True,
                )
                he = nodep.tile([128, NODE_PAIR], f32, tag="he_node")
                nc.scalar.activation(he[:], ps[:], AFUNC, bias=b1n[:, db:db + 1])
                hes.append(he)
            pa2 = ps_pa.tile([128, 2], f32, tag="pa2")
            for t in range(NODE_PAIR // 128):
                for db in range(2):
                    nc.tensor.matmul(
                        pa2[:, t:t + 1],
                        hes[db][:, t * 128:(t + 1) * 128],
                        W2n[:, db:db + 1],
                        start=(db == 0), stop=(db == 1),
                    )
            nc.scalar.activation(pa_sb[:, jp * 2:jp * 2 + 2], pa2[:], AF.Copy)

        wn_f = const.tile([128, NTC], f32)
        nc.vector.scalar_tensor_tensor(
            wn_f[:], pa_sb[:], b2[:, 1:2], sN[:], OP.add, OP.mult,
        )
        wn = const.tile([128, NTC], f32)
        nc.vector.tensor_tensor(wn[:], wn_f[:], hN[:], OP.add)

        NCUM = const.tile([128, NTC, 4], f32)
        nc.vector.tensor_tensor(
            NCUM[:],
            idblk[:].unsqueeze(2).broadcast_to([128, NTC, 4]),
            BrowL[:].unsqueeze(1).broadcast_to([128, NTC, 4]),
            OP.is_lt,
        )
        Yn_ps = ps_acc.tile([4, 1], f32)
        for j in range(NTC):
            nc.tensor.matmul(
                Yn_ps[:], NCUM[:, j, :], wn[:, j:j + 1],
                start=(j == 0), stop=(j == NTC - 1),
            )
        ysb = const.tile([4, 1], f32)
        nc.vector.tensor_copy(ysb[:], Yn_ps[:])
        nc.sync.dma_start(out_d.ap(), ysb[:])

    nc.compile()
    return nc


def _shard(inputs):
    """Host-side sharding. Returns (ET, NT, in_maps, bounds)."""
    node_feats = np.ascontiguousarray(inputs["node_feats"], dtype=np.float32)
    edge_feats = np.ascontiguousarray(inputs["edge_feats"], dtype=np.float32)
    Z = np.asarray(inputs["atomic_numbers"], dtype=np.int32)
    idx_s = np.asarray(inputs["idx_s"], dtype=np.int32)
    idx_t = np.asarray(inputs["idx_t"], dtype=np.int32)
    batch = np.asarray(inputs["batch"], dtype=np.int32)

    bounds = np.searchsorted(batch, np.arange(NUM_GRAPHS + 1)).astype(np.int64)
    g_t = batch[idx_t]
    core_of_edge = np.minimum(g_t // 4, NCORES - 1).astype(np.int32)

    # uniform padded sizes across cores
    e_counts = np.bincount(core_of_edge, minlength=NCORES)
    ET = int(-(-e_counts.max() // EDGE_BLOCK) * EDGE_BLOCK)
    n_counts = bounds[4 * np.arange(NCORES) + 4] - bounds[4 * np.arange(NCORES)]
    NT = int(-(-n_counts.max() // NODE_PAIR) * NODE_PAIR)

    Zext = np.concatenate([Z, [NZ]]).astype(np.int32)
    ascale_ext = np.zeros(NZ + 1, np.float32)
    ascale_ext[:NZ] = np.asarray(inputs["atom_scales"], np.float32)[:, 0]
    ashift_ext = np.zeros(NZ + 1, np.float32)
    ashift_ext[:NZ] = np.asarray(inputs["atom_shifts"], np.float32)[:, 0]
    # shipped transposed: pair_T[b, a] = pair_scales[a*101+b]
    pair = np.ascontiguousarray(
        np.asarray(inputs["pair_scales"], np.float32)[:, 0].reshape(NZ, NZ).T
    )
    W1e = np.ascontiguousarray(inputs["W1e"], np.float32)
    b1e = np.ascontiguousarray(np.asarray(inputs["b1e"], np.float32).reshape(128, 1))
    W2e = np.ascontiguousarray(np.asarray(inputs["W2e"], np.float32).reshape(128, 1))
    W1n = np.ascontiguousarray(inputs["W1n"], np.float32)
    b1n = np.ascontiguousarray(
        np.asarray(inputs["b1n"], np.float32).reshape(2, 128).T
    )
    W2n = np.ascontiguousarray(
        np.asarray(inputs["W2n"], np.float32).reshape(2, 128).T
    )
    b2 = np.tile(np.array(
        [[np.asarray(inputs["b2e"], np.float32)[0],
          np.asarray(inputs["b2n"], np.float32)[0]]], np.float32
    ), (128, 1))

    order = np.argsort(core_of_edge, kind="stable")

    in_maps = []
    for k in range(NCORES):
        n0 = int(bounds[4 * k])
        n1 = int(bounds[4 * k + 4])
        nn = n1 - n0
        sel = order[np.searchsorted(core_of_edge, k, side="left", sorter=order):
                    np.searchsorted(core_of_edge, k, side="right", sorter=order)]
        E = sel.size

        eTk = np.zeros((D_EDGE, ET), np.float32)
        eTk[:, :E] = edge_feats[sel].T
        eis = np.full(ET, SENT_NODE, np.int32)
        eis[:E] = idx_s[sel]
        eit = np.full(ET, SENT_NODE, np.int32)
        eit[:E] = idx_t[sel]
        iswk = np.ascontiguousarray(eis.reshape(ET // 128, 128).T)
        itwk = np.ascontiguousarray(eit.reshape(ET // 128, 128).T)

        nTk = np.zeros((D_NODE, NT), np.float32)
        nTk[:, :nn] = node_feats[n0:n1].T
        Znk = np.full(NT, NZ, np.int32)
        Znk[:nn] = Z[n0:n1]
        Znk = np.ascontiguousarray(Znk.reshape(NT // 128, 128).T)

        Brow = bounds[[4 * k + 1, 4 * k + 2, 4 * k + 3, 4 * k + 4]].astype(np.int32)
        in_maps.append({
            "eT": eTk,
            "nTa": np.ascontiguousarray(nTk[:128]),
            "nTb": np.ascontiguousarray(nTk[128:]),
            "isw": iswk, "itw": itwk, "Zn": Znk,
            "Zext": Zext, "ascale": ascale_ext, "ashift": ashift_ext,
            "pair": pair,
            "W1e": W1e, "b1e": b1e, "W2e": W2e,
            "W1n": W1n, "b1n": b1n, "W2n": W2n, "b2": b2,
            "Brow": np.tile(Brow.reshape(1, 4), (128, 1)),
            "ascR": np.tile(ascale_ext[None, :], (128, 1)),
            "ashR": np.tile(ashift_ext[None, :], (128, 1)),
            "iotaR": np.tile(np.arange(NZ + 1, dtype=np.int32)[None, :], (128, 1)),
            "idn": np.ascontiguousarray(
                (np.arange(NT, dtype=np.int32).reshape(NT // 128, 128).T)),
            "BrowL": np.tile((Brow - n0).reshape(1, 4).astype(np.int32), (128, 1)),
        })
    return ET, NT, in_maps


LAST_RES = None
LAST_RES_NODE = None

_EDGE_KEYS = ["eT", "isw", "itw", "Zext", "ascale", "ashift", "pair", "iotaR",
              "W1e", "b1e", "W2e", "b2", "Brow"]
_NODE_KEYS = ["nTa", "nTb", "Zn", "ascR", "ashR", "iotaR",
              "W1n", "b1n", "W2n", "b2", "idn", "BrowL"]


def kernel(**inputs) -> np.ndarray:
    global LAST_RES, LAST_RES_NODE
    from concourse.bass_utils import run_bass_kernel_spmd

    ET, NT, in_maps = _shard(inputs)
    key = (ET, NT)
    if key not in _CACHE:
        _CACHE[key] = (_build_edge(ET, NT), _build_node(NT))
    nc_e, nc_n = _CACHE[key]

    edge_maps = [{k: m[k] for k in _EDGE_KEYS} for m in in_maps]
    node_maps = [{k: m[k] for k in _NODE_KEYS} for m in in_maps]
    res_e = run_bass_kernel_spmd(nc_e, edge_maps, core_ids=list(range(NCORES)))
    res_n = run_bass_kernel_spmd(nc_n, node_maps, core_ids=list(range(NCORES)))
    LAST_RES = res_e
    LAST_RES_NODE = res_n
    Y = np.zeros(NUM_GRAPHS, np.float32)
    for k in range(NCORES):
        yp = (np.asarray(res_e.results[k]["out"]).reshape(4)
              + np.asarray(res_n.results[k]["out"]).reshape(4))
        Y[4 * k] = yp[0]
        Y[4 * k + 1] = yp[1] - yp[0]
        Y[4 * k + 2] = yp[2] - yp[1]
        Y[4 * k + 3] = yp[3] - yp[2]
    return Y



# revision 15
# speedup vs baseline: 13.8603x; 13.8603x over previous
"""Trainium2 Bass kernel for AllegroScalarOutputHead (segment_reduce).

Strategy (8 NeuronCores, SPMD, no collectives):
  - Graphs 4k..4k+3 -> core k (batch is sorted => contiguous node range).
    Edges go to the core that owns their TARGET node.
  - Features shipped transposed in f16 (halves HBM traffic; 1 cyc/row PE).
  - Host precomputes per-edge coefficient c_e = pair_scales[zs*101+zt] *
    atom_scales[zt] and per-node scale/shift lookups (tiny O(E) table reads;
    the TRN2 DGE only supports >=256B row gathers, so elementwise device
    gathers are impractical). All MLP FLOPs and reductions run on device.
  - edge MLP: mm1 = W1e @ x as 2x[128,512] streams per PSUM pair; mm2 =
    W2e^T @ he as [32,512] replicated rows into PSUM quadrants {0,32,64,96}
    x 4 banks (16-supertile sweeps). One contiguous DVE copy moves the sweep
    to SBUF; one SBUF->SBUF DMA re-partitions rows {0,32,64,96} into a
    [128,64] block of the group's u-tile (so vector work uses all lanes).
  - Per-graph reduction: cumulative is_lt masks vs the 4 graph node-id
    boundaries, mask-multiply-reduce into a [128,4] accumulator, one
    final matmul with ones -> [4,1]; host un-diffs and concatenates.
"""

import numpy as np

NCORES = 8
N_NODES = 50000
NUM_GRAPHS = 32
NZ = 101             # atomic-number entries (0..100)
D_NODE = 256
D_EDGE = 128
SUPER = 512          # supertile (matmul moving columns)
UNIT = 4 * SUPER     # pad granularity
SWEEP = 16 * SUPER   # mm2 psum sweep: 16 supertiles = 8192 slots
GROUP = 8 * SWEEP    # u-tile group: 65536 slots
PAD_I = np.int32(1 << 30)

_CACHE = {}


def _sweep_layout(arr_flat, nsw):
    """[nsw*8192] -> [128, nsw*64]: slot n of sweep s -> (n//64, 64*s + n%64)."""
    return np.ascontiguousarray(
        arr_flat.reshape(nsw, 128, 64).transpose(1, 0, 2).reshape(128, nsw * 64)
    )


def _group_layout(arr_flat, ngrp):
    """[ngrp*65536] -> [ngrp*128, 512]: group g rows [128g, 128g+128) hold the
    sweep layout of its 8 sweeps (slot n of sweep s -> (n//64, 64*s + n%64))."""
    return np.ascontiguousarray(
        arr_flat.reshape(ngrp, 8, 128, 64).transpose(0, 2, 1, 3)
        .reshape(ngrp * 128, 512)
    )


def _build(ET, NT):
    """Single merged SPMD program. ET/NT = padded edges/nodes per core."""
    import concourse.bass as bass
    import concourse.tile as tile
    from concourse import bacc, mybir
    from contextlib import ExitStack

    f32 = mybir.dt.float32
    f16 = mybir.dt.float16
    i32 = mybir.dt.int32
    AF = mybir.ActivationFunctionType
    OP = mybir.AluOpType

    S = ET // SUPER                 # edge supertiles
    NGRP = -(-ET // GROUP)          # edge u-tile groups
    NS = NT // SUPER                # node supertiles
    NSW = -(-NS // 16)              # node sweeps
    NC2 = NSW * 64                  # node u-tile columns
    assert S % 4 == 0 and NS % 4 == 0

    nc = bacc.Bacc("TRN2", debug=False, num_devices=NCORES)

    # ---------------- DRAM parameters --------------------------------------
    eT = nc.declare_dram_parameter("eT", [D_EDGE, ET], f16, isOutput=False)
    CL = nc.declare_dram_parameter("CL", [NGRP * 128, SUPER], f32, isOutput=False)
    itwL = nc.declare_dram_parameter("itwL", [NGRP * 128, SUPER], i32, isOutput=False)
    Brow_d = nc.declare_dram_parameter("Brow", [128, 4], i32, isOutput=False)
    nTa_d = nc.declare_dram_parameter("nTa", [128, NT], f16, isOutput=False)
    nTb_d = nc.declare_dram_parameter("nTb", [128, NT], f16, isOutput=False)
    AL = nc.declare_dram_parameter("AL", [128, NC2], f32, isOutput=False)
    HL = nc.declare_dram_parameter("HL", [128, NC2], f32, isOutput=False)
    idnL = nc.declare_dram_parameter("idnL", [128, NC2], i32, isOutput=False)
    BrowL_d = nc.declare_dram_parameter("BrowL", [128, 4], i32, isOutput=False)
    W1e_d = nc.declare_dram_parameter("W1e", [128, 128], f16, isOutput=False)
    b1e_d = nc.declare_dram_parameter("b1e", [128, 1], f32, isOutput=False)
    W2e_d = nc.declare_dram_parameter("W2e", [128, 32], f16, isOutput=False)
    W1n_d = nc.declare_dram_parameter("W1n", [256, 256], f16, isOutput=False)
    b1n_d = nc.declare_dram_parameter("b1n", [128, 2], f32, isOutput=False)
    W2n_d = nc.declare_dram_parameter("W2n", [128, 64], f16, isOutput=False)
    b2_d = nc.declare_dram_parameter("b2", [128, 2], f32, isOutput=False)  # [b2e,b2n]
    out_d = nc.declare_dram_parameter("out", [4, 1], f32, isOutput=True)

    with tile.TileContext(nc) as tc, ExitStack() as ctx:
        const = ctx.enter_context(tc.tile_pool(name="const", bufs=1))
        xep = ctx.enter_context(tc.tile_pool(name="xep", bufs=3))
        hep = ctx.enter_context(tc.tile_pool(name="hep", bufs=3))
        up = ctx.enter_context(tc.tile_pool(name="up", bufs=2))
        stp = ctx.enter_context(tc.tile_pool(name="stp", bufs=2))
        gscr = ctx.enter_context(tc.tile_pool(name="gscr", bufs=2))
        ps_mm1 = ctx.enter_context(tc.tile_pool(name="ps_mm1", bufs=2, space="PSUM"))
        ps_mm2 = ctx.enter_context(tc.tile_pool(name="ps_mm2", bufs=1, space="PSUM"))

        # ---------------- constants ----------------------------------------
        W1e = const.tile([128, 128], f16)
        nc.sync.dma_start(W1e[:], W1e_d.ap())
        b1e = const.tile([128, 1], f32)
        nc.sync.dma_start(b1e[:], b1e_d.ap())
        W2e = const.tile([128, 32], f16)
        nc.sync.dma_start(W2e[:], W2e_d.ap())
        b2 = const.tile([128, 2], f32)
        nc.sync.dma_start(b2[:], b2_d.ap())
        Brow = const.tile([128, 4], i32)
        nc.sync.dma_start(Brow[:], Brow_d.ap())
        BrowL = const.tile([128, 4], i32)
        nc.sync.dma_start(BrowL[:], BrowL_d.ap())
        W1n = []
        for kb in range(2):
            for db in range(2):
                t = const.tile([128, 128], f16, name=f"w1n{kb}{db}")
                nc.sync.dma_start(
                    t[:], W1n_d.ap()[kb * 128:(kb + 1) * 128, db * 128:(db + 1) * 128]
                )
                W1n.append(t)
        b1n = const.tile([128, 2], f32)
        nc.sync.dma_start(b1n[:], b1n_d.ap())
        W2n = const.tile([128, 64], f16)
        nc.sync.dma_start(W2n[:], W2n_d.ap())
        ones_col = const.tile([128, 1], f32)
        nc.vector.memset(ones_col[:], 1.0)

        accE = const.tile([128, 4], f32)
        nc.vector.memset(accE[:], 0.0)
        accN = const.tile([128, 4], f32)
        nc.vector.memset(accN[:], 0.0)

        # ---------------- edge stream --------------------------------------
        # mm2 sweep: 16 supertiles -> one [128, 2048] 4-bank psum tile; slot
        # r = 4q+b -> [32q:32q+32, 512b:512b+512] (rows replicated 32x).
        # DVE copies the sweep to SBUF; a strided SBUF->SBUF DMA picks rows
        # {0,32,64,96} (flat: 16x512 slot-major) into u-tile cols
        # [64sw, 64sw+64) as [128, 64] row-major (slot n -> (n//64, n%64)).
        XB = 4096  # xe block columns
        utile = ctile = ititle = pt2 = None
        rows = 0
        for s in range(S):
            g, sg = divmod(s, 128)          # u-group, supertile-in-group
            sw, r = divmod(s, 16)           # sweep, slot-in-sweep

            if s % (XB // SUPER) == 0:      # new xe block
                bsz = min(XB, ET - s * SUPER)
                xe = xep.tile([128, XB], f16, tag="xe")
                nc.sync.dma_start(
                    xe[:, 0:bsz], eT.ap()[:, s * SUPER:s * SUPER + bsz]
                )
            if sg == 0:                     # new group: u/c/itw tiles
                rows = min(128, S - s)      # supertiles in this group
                utile = up.tile([128, SUPER], f32, tag="u")
                ctile = up.tile([128, SUPER], f32, tag="c")
                ititle = up.tile([128, SUPER], i32, tag="it")
                if rows < 128:
                    nc.vector.memset(utile[:], 0.0)
                nc.sync.dma_start(ctile[:], CL.ap()[g * 128:g * 128 + 128, :])
                nc.sync.dma_start(ititle[:], itwL.ap()[g * 128:g * 128 + 128, :])

            if s % 2 == 0:                  # mm1 pair
                ps1 = ps_mm1.tile([128, 1024], f32, tag="mm1")
                co = (s * SUPER) % XB
                nc.tensor.matmul(ps1[:, 0:512], W1e[:], xe[:, co:co + 512],
                                 start=True, stop=True)
                nc.tensor.matmul(ps1[:, 512:1024], W1e[:], xe[:, co + 512:co + 1024],
                                 start=True, stop=True)
                he = hep.tile([128, 1024], f16, tag="he")
                nc.scalar.activation(he[:], ps1[:], AF.Silu, bias=b1e[:])
            if r == 0:
                pt2 = ps_mm2.tile([128, 2048], f32, tag="mm2")
                if S - sw * 16 < 16:        # partial sweep: zero unused slots
                    nc.vector.memset(pt2[:], 0.0)
            q, bk = divmod(r, 4)
            nc.tensor.matmul(pt2[32 * q:32 * q + 32, 512 * bk:512 * bk + 512],
                             W2e[:], he[:, (s % 2) * 512:(s % 2) * 512 + 512],
                             start=True, stop=True, tile_position=(0, 32 * q))
            if r == 15 or s == S - 1:       # sweep done: copy + re-partition
                stag = stp.tile([128, 2048], f32, tag="stag")
                nc.vector.tensor_copy(stag[:], pt2[:])
                uc = (sw % 8) * 64
                nc.sync.dma_start(utile[:, uc:uc + 64], stag[0:128:32, :])

            if sg == 127 or s == S - 1:     # group done: apply c + masks
                um = gscr.tile([128, SUPER], f32, tag="um")
                nc.vector.scalar_tensor_tensor(
                    um[:], utile[:], b2[:, 0:1], ctile[:], OP.add, OP.mult
                )
                M4 = gscr.tile([128, 4, SUPER], f32, tag="m4")
                nc.vector.tensor_tensor(
                    M4[:],
                    ititle[:].unsqueeze(1).broadcast_to([128, 4, SUPER]),
                    Brow[:].unsqueeze(2).broadcast_to([128, 4, SUPER]),
                    OP.is_lt,
                )
                zz = gscr.tile([128, 4, SUPER], f32, tag="zz")
                nc.vector.tensor_tensor(
                    zz[:], um[:].unsqueeze(1).broadcast_to([128, 4, SUPER]),
                    M4[:], OP.mult,
                )
                racc = gscr.tile([128, 4], f32, tag="racc")
                nc.vector.tensor_reduce(
                    racc[:].unsqueeze(2), zz[:], mybir.AxisListType.X, OP.add
                )
                nc.vector.tensor_tensor(accE[:], accE[:], racc[:], OP.add)

        # ---------------- node stream --------------------------------------
        nTa = const.tile([128, NT], f16)
        nc.sync.dma_start(nTa[:], nTa_d.ap())
        nTb = const.tile([128, NT], f16)
        nc.sync.dma_start(nTb[:], nTb_d.ap())
        untile = const.tile([128, NC2], f32)
        atile = const.tile([128, NC2], f32)
        nc.sync.dma_start(atile[:], AL.ap())
        htile = const.tile([128, NC2], f32)
        nc.sync.dma_start(htile[:], HL.ap())
        intile = const.tile([128, NC2], i32)
        nc.sync.dma_start(intile[:], idnL.ap())

        pt2n = None
        hn = [None, None]
        for j in range(NS):
            sw, r = divmod(j, 16)
            if j % 2 == 0:                  # 2-supertile pair, both halves
                for db in range(2):
                    psn = ps_mm1.tile([128, 1024], f32, tag="mm1")
                    for j2 in range(2):
                        c2 = slice((j + j2) * SUPER, (j + j2 + 1) * SUPER)
                        nc.tensor.matmul(psn[:, j2 * 512:j2 * 512 + 512],
                                         W1n[0 * 2 + db][:], nTa[:, c2],
                                         start=True, stop=False)
                        nc.tensor.matmul(psn[:, j2 * 512:j2 * 512 + 512],
                                         W1n[1 * 2 + db][:], nTb[:, c2],
                                         start=False, stop=True)
                    h = hep.tile([128, 1024], f16, tag="he")
                    nc.scalar.activation(h[:], psn[:], AF.Silu, bias=b1n[:, db:db + 1])
                    hn[db] = h
            if r == 0:
                pt2n = ps_mm2.tile([128, 2048], f32, tag="mm2")
                if NS - sw * 16 < 16:
                    nc.vector.memset(pt2n[:], 0.0)
            q, bk = divmod(r, 4)
            sl = pt2n[32 * q:32 * q + 32, 512 * bk:512 * bk + 512]
            hcols = slice((j % 2) * 512, (j % 2) * 512 + 512)
            nc.tensor.matmul(sl, W2n[:, 0:32], hn[0][:, hcols],
                             start=True, stop=False, tile_position=(0, 32 * q))
            nc.tensor.matmul(sl, W2n[:, 32:64], hn[1][:, hcols],
                             start=False, stop=True, tile_position=(0, 32 * q))
            if r == 15 or j == NS - 1:
                stag = stp.tile([128, 2048], f32, tag="stag")
                nc.vector.tensor_copy(stag[:], pt2n[:])
                uc = sw * 64
                nc.sync.dma_start(untile[:, uc:uc + 64], stag[0:128:32, :])

        # wn = (pe_n + b2n) * ascale[z] + ashift[z]
        wn1 = gscr.tile([128, NC2], f32, tag="um")
        nc.vector.scalar_tensor_tensor(
            wn1[:], untile[:], b2[:, 1:2], atile[:], OP.add, OP.mult
        )
        wn = gscr.tile([128, NC2], f32, tag="wn")
        nc.vector.tensor_tensor(wn[:], wn1[:], htile[:], OP.add)
        M4n = gscr.tile([128, 4, NC2], f32, tag="m4n")
        nc.vector.tensor_tensor(
            M4n[:],
            intile[:].unsqueeze(1).broadcast_to([128, 4, NC2]),
            BrowL[:].unsqueeze(2).broadcast_to([128, 4, NC2]),
            OP.is_lt,
        )
        zzn = gscr.tile([128, 4, NC2], f32, tag="zzn")
        nc.vector.tensor_tensor(
            zzn[:], wn[:].unsqueeze(1).broadcast_to([128, 4, NC2]),
            M4n[:], OP.mult,
        )
        raccn = gscr.tile([128, 4], f32, tag="racc")
        nc.vector.tensor_reduce(
            raccn[:].unsqueeze(2), zzn[:], mybir.AxisListType.X, OP.add
        )
        nc.vector.tensor_tensor(accN[:], accN[:], raccn[:], OP.add)

        # ---------------- finalize -----------------------------------------
        accT = const.tile([128, 4], f32)
        nc.vector.tensor_tensor(accT[:], accE[:], accN[:], OP.add)
        Yps = ps_mm1.tile([4, 1], f32, tag="mm1")
        nc.tensor.matmul(Yps[:], accT[:], ones_col[:], start=True, stop=True)
        ysb = const.tile([4, 1], f32)
        nc.vector.tensor_copy(ysb[:], Yps[:])
        nc.sync.dma_start(out_d.ap(), ysb[:])

    nc.compile()
    return nc


def _shard(inputs):
    f16 = np.float16

    node_feats = np.asarray(inputs["node_feats"], np.float32)
    edge_feats = np.asarray(inputs["edge_feats"], np.float32)
    Z = np.asarray(inputs["atomic_numbers"], np.int64)
    idx_s = np.asarray(inputs["idx_s"], np.int32)
    idx_t = np.asarray(inputs["idx_t"], np.int32)
    batch = np.asarray(inputs["batch"], np.int32)

    bounds = np.searchsorted(batch, np.arange(NUM_GRAPHS + 1)).astype(np.int64)
    g_t = batch[idx_t]
    core_of_edge = (g_t >> 2).astype(np.int32)

    e_counts = np.bincount(core_of_edge, minlength=NCORES)
    ET = int(-(-e_counts.max() // UNIT) * UNIT)
    n_counts = bounds[4 * np.arange(NCORES) + 4] - bounds[4 * np.arange(NCORES)]
    NT = int(-(-n_counts.max() // UNIT) * UNIT)
    NGRP = -(-ET // GROUP)
    NSW = -(-(NT // SUPER) // 16)

    ascale = np.asarray(inputs["atom_scales"], np.float32)[:, 0]
    ashift = np.asarray(inputs["atom_shifts"], np.float32)[:, 0]
    pair = np.asarray(inputs["pair_scales"], np.float32)[:, 0]

    W1e = np.asarray(inputs["W1e"], np.float32).astype(f16)
    b1e = np.asarray(inputs["b1e"], np.float32).reshape(128, 1)
    W2e = np.tile(np.asarray(inputs["W2e"], np.float32).reshape(128, 1),
                  (1, 32)).astype(f16)
    W1n = np.asarray(inputs["W1n"], np.float32).astype(f16)
    b1n = np.ascontiguousarray(np.asarray(inputs["b1n"], np.float32).reshape(2, 128).T)
    W2n_2 = np.asarray(inputs["W2n"], np.float32).reshape(2, 128).T
    W2n = np.concatenate(
        [np.tile(W2n_2[:, 0:1], (1, 32)), np.tile(W2n_2[:, 1:2], (1, 32))], axis=1
    ).astype(f16)
    b2 = np.tile(np.array(
        [[np.asarray(inputs["b2e"], np.float32)[0],
          np.asarray(inputs["b2n"], np.float32)[0]]], np.float32), (128, 1))

    # per-edge coefficient (host table lookup; see module docstring)
    c_all = (pair[Z[idx_s] * NZ + Z[idx_t]] * ascale[Z[idx_t]]).astype(np.float32)

    order = np.argsort(core_of_edge, kind="stable")
    starts = np.searchsorted(core_of_edge, np.arange(NCORES + 1), sorter=order)

    in_maps = []
    for k in range(NCORES):
        n0 = int(bounds[4 * k])
        n1 = int(bounds[4 * k + 4])
        nn = n1 - n0
        sel = order[starts[k]:starts[k + 1]]
        E = sel.size

        eTk = np.zeros((D_EDGE, ET), f16)
        eTk[:, :E] = edge_feats[sel].T
        cpad = np.zeros(NGRP * GROUP, np.float32)
        cpad[:E] = c_all[sel]
        itw = np.full(NGRP * GROUP, PAD_I, np.int32)
        itw[:E] = idx_t[sel]

        nTk = np.zeros((D_NODE, NT), f16)
        nTk[:, :nn] = node_feats[n0:n1].T
        NTW = NSW * SWEEP
        apad = np.zeros(NTW, np.float32)
        apad[:nn] = ascale[Z[n0:n1]]
        hpad = np.zeros(NTW, np.float32)
        hpad[:nn] = ashift[Z[n0:n1]]
        idn = np.full(NTW, PAD_I, np.int32)
        idn[:nn] = np.arange(nn, dtype=np.int32)

        Brow = bounds[[4 * k + 1, 4 * k + 2, 4 * k + 3, 4 * k + 4]].astype(np.int32)
        in_maps.append({
            "eT": eTk,
            "CL": _group_layout(cpad, NGRP),
            "itwL": _group_layout(itw, NGRP),
            "Brow": np.tile(Brow.reshape(1, 4), (128, 1)),
            "nTa": np.ascontiguousarray(nTk[:128]),
            "nTb": np.ascontiguousarray(nTk[128:]),
            "AL": _sweep_layout(apad, NSW),
            "HL": _sweep_layout(hpad, NSW),
            "idnL": _sweep_layout(idn, NSW),
            "BrowL": np.tile((Brow - n0).reshape(1, 4), (128, 1)),
            "W1e": W1e, "b1e": b1e, "W2e": W2e,
            "W1n": W1n, "b1n": b1n, "W2n": W2n, "b2": b2,
        })
    return ET, NT, in_maps


LAST_RES = None


def kernel(**inputs) -> np.ndarray:
    global LAST_RES
    from concourse.bass_utils import run_bass_kernel_spmd

    ET, NT, in_maps = _shard(inputs)
    key = (ET, NT)
    if key not in _CACHE:
        _CACHE[key] = _build(ET, NT)
    nc = _CACHE[key]

    res = run_bass_kernel_spmd(nc, in_maps, core_ids=list(range(NCORES)))
    LAST_RES = res
    Y = np.zeros(NUM_GRAPHS, np.float32)
    for k in range(NCORES):
        yp = np.asarray(res.results[k]["out"]).reshape(4)
        Y[4 * k] = yp[0]
        Y[4 * k + 1] = yp[1] - yp[0]
        Y[4 * k + 2] = yp[2] - yp[1]
        Y[4 * k + 3] = yp[3] - yp[2]
    return Y
